# revision 22
# baseline (speedup 1.0000x reference)
"""Single-head attention on 8 TRN2 NeuronCores — data-parallel over batch.

Reference (per batch element b):
    q = x @ Wq.T + bq; k = x @ Wk.T + bk; v = x @ Wv.T + bv     [S, D]
    scores = q @ k.T / sqrt(S); masked where attention_mask==0
    out = softmax(scores) @ v                                    [S, D]

Shapes: B=8, S=2048, DIN=1024, D=128.  Core i computes batch element i.

Key optimizations over the straightforward version:
  * Key compaction: softmax is permutation-invariant over keys and ~half the
    keys are masked out.  Host-side we gather the unmasked keys and pad to a
    fixed M_pad (multiple of 128).  scores/exp/context then run on M_pad keys
    instead of S.  Padded keys get v=0 (zeroed via mask multiply) and are
    excluded from the softmax denominator by using the 0/1 mask column as the
    stationary operand of the denominator matmuls.
  * fp8 (e4m3) x and 16*Wq for the q projection halve that DMA; the 1/16
    rescale is folded into the bias-apply copy.  The k/v path stays bf16 —
    the near-uniform softmax averages ~1000 v rows with heavy cancellation,
    so fp8 noise in v does not shrink relative to the output.
  * Input DMAs are spread across the sync/gpsimd/vector/scalar queues and
    sized large, since each dma_start costs ~0.7us of issue time on its queue.
  * The middle loop is split into two query halves (i in [0,1024), [1024,2048))
    so the context accumulator fits in 2 PSUM banks and the first half's
    normalize/store tail overlaps the second half's sweep.  Consumer matmuls
    (denominator + context) of step jt are emitted after scores of step jt+1
    so the in-order PE never idles waiting for exp.
  * Denominator matmuls (M=1) for the 2 query chunks of a half are packed into
    different 32-column groups of the PE array via tile_position, sharing one
    PSUM bank across all 4 (half, chunk) pairs.
  * PE warm-up matmuls during the initial DMA engage the HAM clock-gate
    (1.2 -> 2.4 GHz) before the real work starts; more are interleaved into
    the DMA-paced k projection to keep it engaged.
"""

import numpy as np
import ml_dtypes

B, S, DIN, DOUT = 8, 2048, 1024, 128
N_CORES = 8
NCH = DIN // 128          # 8 contraction chunks
BF16 = ml_dtypes.bfloat16
FP8 = ml_dtypes.float8_e4m3fn
SCALE = 1.0 / float(np.sqrt(S))
WSCALE = 16.0             # Wq shipped as 16*Wq in fp8; undone in bias copy

_CACHED = {}


def _blocks(total, maxw=512):
    out = []
    off = 0
    while off < total:
        w = min(maxw, total - off)
        out.append((off, w))
        off += w
    return out


def _build(m_pad):
    import concourse.bacc as bacc
    import concourse.mybir as mybir
    from concourse.tile import TileContext

    dt = mybir.dt
    F32, BF, F8 = dt.float32, dt.bfloat16, dt.float8e4
    Exp = mybir.ActivationFunctionType.Exp
    Ident = mybir.ActivationFunctionType.Identity
    Copy = mybir.ActivationFunctionType.Copy
    MUL, ADD = mybir.AluOpType.mult, mybir.AluOpType.add

    nkt = m_pad // 128            # key tiles
    kblk = _blocks(m_pad)         # column blocks for the k/v projections
    # blob32: bq | bk | (unused) | mcolf | identf | onesrow(128) | bvrow(128)
    nb32 = 3 + nkt + 128 + 256

    nc = bacc.Bacc("TRN2", target_bir_lowering=False)

    # xq[p, ((sb*8+c)*512)+s'] = x[sb*512+s', c*128+p], 4 slabs of s
    xq = [nc.declare_dram_parameter(f"xq{sb}", [128, NCH * 512], F8, False)
          for sb in range(4)]
    # xkv2[p2][p, i*m_pad + j'] = x[idx[j'], (2*p2+i)*128+p]
    xkv = [nc.declare_dram_parameter(f"xkv{p2}", [128, 2 * m_pad], BF, False)
           for p2 in range(4)]
    wkv = nc.declare_dram_parameter("wkv", [128, 2 * NCH * 128], BF, False)
    wq = nc.declare_dram_parameter("wq", [128, NCH * 128], F8, False)
    blob32 = nc.declare_dram_parameter("blob32", [128, nb32], F32, False)
    out = nc.declare_dram_parameter("out", [S, DOUT], F32, True)
    import os
    _dbg = os.environ.get("KDBG")
    if _dbg:
        dbgv = nc.declare_dram_parameter("dbgv", [128, m_pad], F32, True)
        dbgk = nc.declare_dram_parameter("dbgk", [128, m_pad], F32, True)
        dbgq = nc.declare_dram_parameter("dbgq", [128, S], F32, True)

    with TileContext(nc) as tc:
        with (
            tc.tile_pool(name="const", bufs=1) as cp,
            tc.tile_pool(name="work", bufs=1) as wp,
            tc.tile_pool(name="et", bufs=3) as ep,
            tc.tile_pool(name="ctxs", bufs=2) as xp,
            tc.tile_pool(name="io", bufs=4) as iop,
        ):
            # ---- warm scratch (no DMA dependency) ----
            warm = wp.tile([128, 512], BF, tag="warm")
            nc.gpsimd.memset(warm[:], 0.0)
            warmf = wp.tile([128, 16], F32, tag="warmf")
            nc.gpsimd.memset(warmf[:], 0.0)
            # Both HW DGE queues (sync + scalar) pull the k/v-side
            # activations first, interleaved in the order the k/v projection
            # consumes them; q-side follows.  (gpsimd DMA is a ~90 GB/s
            # software DGE — never used for bulk.)
            xkv_sb = [None] * 4
            xq_sb = [None] * 4

            def dma(eng, name, shape, dtp, dram):
                t = cp.tile(shape, dtp, tag=name)
                eng.dma_start(out=t[:], in_=dram[:])
                return t

            xkv_sb[0] = dma(nc.scalar, "xkv0", [128, 2 * m_pad], BF, xkv[0])
            xq_sb[0] = dma(nc.scalar, "xq0", [128, NCH * 512], F8, xq[0])
            xq_sb[1] = dma(nc.scalar, "xq1", [128, NCH * 512], F8, xq[1])
            xkv_sb[2] = dma(nc.scalar, "xkv2", [128, 2 * m_pad], BF, xkv[2])

            wkv_sb = dma(nc.sync, "wkv", [128, 2 * NCH * 128], BF, wkv)
            wq_sb = dma(nc.sync, "wq", [128, NCH * 128], F8, wq)
            b32_sb = dma(nc.sync, "b32", [128, nb32], F32, blob32)
            xkv_sb[1] = dma(nc.sync, "xkv1", [128, 2 * m_pad], BF, xkv[1])
            xkv_sb[3] = dma(nc.sync, "xkv3", [128, 2 * m_pad], BF, xkv[3])
            xq_sb[2] = dma(nc.sync, "xq2", [128, NCH * 512], F8, xq[2])
            xq_sb[3] = dma(nc.sync, "xq3", [128, NCH * 512], F8, xq[3])

            # exp table load AFTER the scalar-queue DMA issues
            warme = wp.tile([128, 16], F32, tag="warme")
            nc.scalar.activation(warme[:], warmf[:], Exp)   # load exp table

            wk_sb = wkv_sb[:, 0:NCH * 128]
            wv_sb = wkv_sb[:, NCH * 128:2 * NCH * 128]
            bq_sb = b32_sb[:, 0:1]
            bk_sb = b32_sb[:, 1:2]
            bvT_sb = b32_sb[:, 2:3]
            mcolf_sb = b32_sb[:, 3:3 + nkt]
            identf_sb = b32_sb[:, 3 + nkt:3 + nkt + 128]
            onesr_sb = b32_sb[0:1, 3 + nkt + 128:3 + nkt + 256]
            bvr_sb = b32_sb[0:1, 3 + nkt + 256:3 + nkt + 384]
            mcol_sb = wp.tile([128, nkt], BF, tag="mcol")

            qT_sb = wp.tile([128, S], BF, tag="qT")
            kT_sb = wp.tile([128, m_pad], BF, tag="kT")
            v_sb = wp.tile([128, m_pad], BF, tag="v")

            def xkv_ap(c, o, w):
                return xkv_sb[c // 2][:, (c % 2) * m_pad + o:
                                      (c % 2) * m_pad + o + w]

            # ================= prologue =================
            with tc.tile_pool(name="proj", bufs=8, space="PSUM") as pp:
                kps = [pp.tile([128, w], F32, tag="proj", name=f"kps{i}")
                       for i, (o, w) in enumerate(kblk)]
                vps = [pp.tile([128, w], F32, tag="proj", name=f"vps{i}")
                       for i, (o, w) in enumerate(kblk)]
                qps = [pp.tile([128, 512], F32, tag="proj", name=f"qps{i}")
                       for i in range(2)]

                # PE warm-up into qps[0] (reset later by the real qh0
                # start=True): engage HAM while DMAs stream
                for _ in range(4):
                    nc.tensor.matmul(qps[0][:], warm[:, 0:128], warm[:],
                                     start=True, stop=True)
                nc.vector.tensor_copy(mcol_sb[:], mcolf_sb[:])  # bf16 cast
                onesr16 = wp.tile([1, 128], BF, tag="onesr16")
                nc.gpsimd.memset(onesr16[:], 1.0)
                bvr16 = wp.tile([1, 128], BF, tag="bvr16")
                nc.vector.tensor_copy(bvr16[:], bvr_sb)   # bf16 cast

                def kv_chunk(c):
                    # kT[d, j'] += wk_c.T @ x_c ; v[j', d] += x_c.T @ wv_c
                    for i, (o, w) in enumerate(kblk):
                        nc.tensor.matmul(
                            kps[i][:], wk_sb[:, c * 128:(c + 1) * 128],
                            xkv_ap(c, o, w),
                            start=(c == 0), stop=(c == NCH - 1))
                    for jt in range(nkt):
                        i, r = divmod(jt, 4)
                        # start only on the first region per PSUM tile: the
                        # lazy zero of start=True covers ALL columns of the
                        # written partitions, so a later region's start would
                        # wipe its neighbors' already-accumulated chunk
                        nc.tensor.matmul(
                            vps[i][:, r * 128:(r + 1) * 128],
                            xkv_ap(c, jt * 128, 128),
                            wv_sb[:, c * 128:(c + 1) * 128],
                            start=(c == 0 and r == 0), stop=False,
                            skip_group_check=True)

                def qh0_group(sb, half):
                    for c in range(4 * half, 4 * half + 4):
                        nc.tensor.matmul(
                            qps[sb][:], wq_sb[:, c * 128:(c + 1) * 128],
                            xq_sb[sb][:, c * 512:(c + 1) * 512],
                            start=(c == 0), stop=(c == NCH - 1))

                kv_chunk(0)
                for _ in range(3):
                    nc.tensor.matmul(qps[1][:], warm[:, 0:128], warm[:],
                                     start=True, stop=True)
                kv_chunk(1)
                for _ in range(3):
                    nc.tensor.matmul(qps[1][:], warm[:, 0:128], warm[:],
                                     start=True, stop=True)
                qh0_group(0, 0)
                qh0_group(0, 1)
                kv_chunk(2)
                kv_chunk(3)
                qh0_group(1, 0)
                qh0_group(1, 1)
                kv_chunk(4)
                kv_chunk(5)
                kv_chunk(6)
                kv_chunk(7)

                # v bias via K=1 outer product ones[j] x bv[d], then the
                # masked psum->sbuf copies; pads get v=0
                for jt in range(nkt):
                    i, r = divmod(jt, 4)
                    nc.tensor.matmul(
                        vps[i][:, r * 128:(r + 1) * 128],
                        onesr16[:], bvr16[:],
                        start=False, stop=True, skip_group_check=True)

                # projections out of PSUM: kT on ACT, q on DVE, v masked on
                # ACT+DVE alternating
                for i, (o, w) in enumerate(kblk):
                    nc.scalar.activation(kT_sb[:, o:o + w], kps[i][:], Ident,
                                         bias=bk_sb)
                    if i == 0:
                        for _ in range(10):
                            nc.tensor.matmul(kps[0][:], warm[:, 0:128],
                                             warm[:], start=True, stop=True)
                for sb in range(2):
                    nc.vector.tensor_scalar(
                        qT_sb[:, sb * 512:(sb + 1) * 512], qps[sb][:],
                        1.0 / WSCALE, bq_sb, MUL, ADD)
                for jt in range(nkt):
                    i, r = divmod(jt, 4)
                    eng = nc.vector if jt % 2 else nc.scalar
                    if jt % 2:
                        nc.vector.tensor_scalar_mul(
                            v_sb[:, jt * 128:(jt + 1) * 128],
                            vps[i][:, r * 128:(r + 1) * 128],
                            mcolf_sb[:, jt:jt + 1])
                    else:
                        nc.scalar.activation(
                            v_sb[:, jt * 128:(jt + 1) * 128],
                            vps[i][:, r * 128:(r + 1) * 128], Copy,
                            scale=mcolf_sb[:, jt:jt + 1])

            if _dbg:
                dv = wp.tile([128, m_pad], F32, tag="dbgv")
                nc.vector.tensor_copy(dv[:], v_sb[:])
                nc.sync.dma_start(out=dbgv[:], in_=dv[:])
                dk = wp.tile([128, m_pad], F32, tag="dbgk")
                nc.vector.tensor_copy(dk[:], kT_sb[:])
                nc.sync.dma_start(out=dbgk[:], in_=dk[:])
                dq = wp.tile([128, S], F32, tag="dbgq")
                nc.vector.tensor_copy(dq[:], qT_sb[:])
                nc.sync.dma_start(out=dbgq[:], in_=dq[:])

            # ================= middle sweep =================
            with (
                tc.tile_pool(name="sp", bufs=2, space="PSUM") as sp,
                tc.tile_pool(name="cpp", bufs=2, space="PSUM") as cpp,
                tc.tile_pool(name="sums", bufs=1, space="PSUM") as smp,
                tc.tile_pool(name="tp", bufs=1, space="PSUM") as tpp,
            ):
                sums_ps = smp.tile([128, 512], F32, tag="sums")
                ss_sb = wp.tile([128, 512], F32, tag="ss")
                rt_sb = wp.tile([128, 2 * 8], F32, tag="rt")
                recip_sb = wp.tile([128, 2 * 8], F32, tag="recip")

                qh1_state = {}

                def qh1_piece(sb, cpair):
                    # second-half q projection, 2 chunks at a time in the tp
                    # bank; bias copy on DVE so ACT keeps streaming exps
                    if sb not in qh1_state:
                        qh1_state[sb] = tpp.tile([128, 512], F32, tag="tp",
                                                 name=f"qh1_{sb}")
                    qp = qh1_state[sb]
                    for c in (2 * cpair, 2 * cpair + 1):
                        nc.tensor.matmul(
                            qp[:], wq_sb[:, c * 128:(c + 1) * 128],
                            xq_sb[sb][:, c * 512:(c + 1) * 512],
                            start=(c == 0), stop=(c == NCH - 1))
                    if cpair == 3:
                        nc.vector.tensor_scalar(
                            qT_sb[:, sb * 512:(sb + 1) * 512], qp[:],
                            1.0 / WSCALE, bq_sb, MUL, ADD)

                def emit_scores(h, jt):
                    ps = sp.tile([128, 1024], F32, tag="sc", name=f"sc{h}_{jt}")
                    for n in range(2):
                        nc.tensor.matmul(
                            ps[:, n * 512:(n + 1) * 512],
                            kT_sb[:, jt * 128:(jt + 1) * 128],
                            qT_sb[:, h * 1024 + n * 512:
                                  h * 1024 + (n + 1) * 512],
                            start=True, stop=True)
                    et = ep.tile([128, 1024], BF, tag="et", name=f"et{h}_{jt}")
                    nc.scalar.activation(et[:], ps[:], Exp, scale=SCALE)
                    return et

                def emit_consume(h, jt, et, ctx_ps):
                    for icg in range(2):
                        base = 64 * h + 32 * icg
                        nc.tensor.matmul(
                            sums_ps[base:base + 1, :],
                            mcol_sb[:, jt:jt + 1],
                            et[:, icg * 512:(icg + 1) * 512],
                            start=(jt == 0), stop=(jt == nkt - 1),
                            tile_position=(0, base),
                            skip_group_check=True)
                    for n in range(2):
                        nc.tensor.matmul(
                            ctx_ps[n][:],
                            v_sb[:, jt * 128:(jt + 1) * 128],
                            et[:, n * 512:(n + 1) * 512],
                            start=(jt == 0), stop=(jt == nkt - 1))

                tail_state = {}

                def tail_stage(h, ctx_ps, stage):
                    """Stage 0: denominator chain + ctx psum drain;
                    1: first 512 ctx cols transposed/normalized/stored;
                    2: second 512.  For h=0 (overlapping the h1 sweep) all
                    copies stay off ACT so exp keeps streaming; for the final
                    h=1 tail both ACT and DVE are used for latency."""
                    st = tail_state.setdefault(h, {})
                    if stage == 0:
                        if h == 0:
                            st['t'] = [(tpp.tile([128, 512], F32, tag="tp",
                                                 name="t0a"), 0)] * 3
                        else:
                            ta = sp.tile([128, 1024], F32, tag="sc", name="t1a")
                            tb = sp.tile([128, 1024], F32, tag="sc", name="t1b")
                            st['t'] = [(ta, 0), (ta, 4), (tb, 0)]
                        # denominator chain first (recip gates the normalizes)
                        if h == 0:
                            nc.vector.tensor_copy(
                                ss_sb[0:64, :], sums_ps[0:64, :])
                        else:
                            nc.scalar.activation(
                                ss_sb[64:128, :], sums_ps[64:128, :], Copy)
                        ctx_sb = xp.tile([128, 1024], F32, tag="ctxT",
                                         name=f"ctxT{h}")
                        st['ctx'] = ctx_sb
                        nc.vector.tensor_copy(ctx_sb[:, 0:512], ctx_ps[0][:])
                        stile, soff = st['t'][0]
                        for b in range(4):
                            tcol = (soff + b) * 128
                            nc.tensor.transpose(
                                stile[:, tcol:tcol + 128],
                                ss_sb[:, b * 128:(b + 1) * 128], identf_sb)
                            for icg in range(2):
                                it = h * 8 + icg * 4 + b
                                srcc = tcol + 64 * h + 32 * icg
                                nc.vector.tensor_copy(
                                    rt_sb[:, it:it + 1],
                                    stile[:, srcc:srcc + 1])
                        nc.vector.reciprocal(
                            recip_sb[:, h * 8:(h + 1) * 8],
                            rt_sb[:, h * 8:(h + 1) * 8])
                        if h == 0:
                            nc.vector.tensor_copy(ctx_sb[:, 512:1024],
                                                  ctx_ps[1][:])
                        else:
                            nc.scalar.activation(ctx_sb[:, 512:1024],
                                                 ctx_ps[1][:], Copy)
                        return
                    icg = stage - 1
                    ctx_sb = st['ctx']
                    ctile, coff = st['t'][1 + icg]
                    o_sb = iop.tile([128, 512], F32, tag="osb",
                                    name=f"osb{h}_{icg}")
                    for b in range(4):
                        tcol = (coff + b) * 128
                        nc.tensor.transpose(
                            ctile[:, tcol:tcol + 128],
                            ctx_sb[:, icg * 512 + b * 128:
                                   icg * 512 + (b + 1) * 128], identf_sb)
                        it = h * 8 + icg * 4 + b
                        if h == 1 and b % 2 == 0:
                            nc.scalar.activation(
                                o_sb[:, b * 128:(b + 1) * 128],
                                ctile[:, tcol:tcol + 128], Copy,
                                scale=recip_sb[:, it:it + 1])
                        else:
                            nc.vector.tensor_scalar_mul(
                                o_sb[:, b * 128:(b + 1) * 128],
                                ctile[:, tcol:tcol + 128],
                                recip_sb[:, it:it + 1])
                    r0 = h * 1024 + icg * 512
                    eng = nc.scalar if (h == 1 and icg == 1) else nc.sync
                    eng.dma_start(
                        out=out[r0:r0 + 512, :].rearrange(
                            "(b p) d -> p b d", b=4),
                        in_=o_sb[:])

                ctx_tiles = {}

                def ctx_tile(h):
                    if h not in ctx_tiles:
                        ctx_tiles[h] = [
                            cpp.tile([128, 512], F32, tag="ctx",
                                     name=f"ctx{h}_{icg}")
                            for icg in range(2)]
                    return ctx_tiles[h]

                for h in range(2):
                    prev = None
                    for jt in range(nkt):
                        et = emit_scores(h, jt)
                        if h == 1 and jt in (1, 2, 3):
                            # h0 tail staged here: its PE work hides in the
                            # exp pipeline-fill bubble of the h1 sweep
                            tail_stage(0, ctx_tile(0), jt - 1)
                        if prev is not None:
                            emit_consume(h, jt - 1, prev, ctx_tile(h))
                        prev = et
                        if h == 0 and jt <= 3:
                            qh1_piece(2, jt)
                        elif h == 0 and 4 <= jt <= 7:
                            qh1_piece(3, jt - 4)
                    emit_consume(h, nkt - 1, prev, ctx_tile(h))
                for stage in range(3):
                    tail_stage(1, ctx_tile(1), stage)

    nc.compile()
    return nc


def _prep_core_inputs(xb, Wq, bq, Wk, bk, Wv, bv, maskb, m_pad):
    """Host-side layout prep for one batch element."""
    nkt = m_pad // 128
    idx = np.nonzero(maskb != 0)[0]
    cnt = len(idx)

    xg = np.zeros((m_pad, DIN), dtype=np.float32)
    xg[:cnt] = xb[idx]

    def chunkT(m, dtp):  # [rows, DIN] -> [128, NCH*rows], m.T chunked over DIN
        mt = np.ascontiguousarray(m.T)          # [DIN, rows]
        c = mt.shape[1]
        return np.ascontiguousarray(
            mt.reshape(NCH, 128, c).transpose(1, 0, 2).reshape(128, NCH * c)
        ).astype(dtp)

    xt = xb.T.reshape(NCH, 128, 4, 512)          # [c, p, sb, s']
    xq4 = np.ascontiguousarray(
        xt.transpose(1, 2, 0, 3).reshape(128, 4 * NCH * 512)).astype(FP8)

    xkvt = xg.T.reshape(NCH, 128, m_pad)         # [c, p, j']
    m01 = np.zeros((m_pad,), dtype=np.float32)
    m01[:cnt] = 1.0
    mt = np.ascontiguousarray(m01.reshape(nkt, 128).T)   # [128, nkt]

    identf = np.eye(128, dtype=np.float32)
    onesrow = np.zeros((128, 128), dtype=np.float32)
    onesrow[0, :] = 1.0
    bvrow = np.zeros((128, 128), dtype=np.float32)
    bvrow[0, :] = bv
    blob32 = np.concatenate([
        bq.reshape(128, 1), bk.reshape(128, 1), bv.reshape(128, 1),
        mt, identf, onesrow, bvrow], axis=1).astype(np.float32)

    d = {
        "wq": chunkT(Wq * WSCALE, FP8),
        "wkv": np.concatenate([chunkT(Wk, BF16), chunkT(Wv, BF16)], axis=1),
        "blob32": np.ascontiguousarray(blob32),
    }
    for sb in range(4):
        d[f"xq{sb}"] = np.ascontiguousarray(
            xq4[:, sb * NCH * 512:(sb + 1) * NCH * 512])
    for p2 in range(4):
        d[f"xkv{p2}"] = np.ascontiguousarray(
            np.concatenate([xkvt[2 * p2], xkvt[2 * p2 + 1]], axis=1)
        ).astype(BF16)
    return d


def kernel(x, Wq, bq, Wk, bk, Wv, bv, attention_mask, _trace=False):
    from concourse.bass_utils import run_bass_kernel_spmd

    x = np.asarray(x, dtype=np.float32)
    Wq = np.asarray(Wq, dtype=np.float32)
    Wk = np.asarray(Wk, dtype=np.float32)
    Wv = np.asarray(Wv, dtype=np.float32)
    bq = np.asarray(bq, dtype=np.float32)
    bk = np.asarray(bk, dtype=np.float32)
    bv = np.asarray(bv, dtype=np.float32)
    mask = np.asarray(attention_mask)

    counts = [int((mask[b, 0] != 0).sum()) for b in range(B)]
    m_pad = min(S, max(512, int(-(-max(counts) // 128) * 128)))

    if ("nc", m_pad) not in _CACHED:
        _CACHED[("nc", m_pad)] = _build(m_pad)
    nc = _CACHED[("nc", m_pad)]

    in_maps = [
        _prep_core_inputs(x[b], Wq, bq, Wk, bk, Wv, bv, mask[b, 0], m_pad)
        for b in range(B)
    ]
    res = run_bass_kernel_spmd(
        nc, in_maps, core_ids=list(range(N_CORES)), trace=_trace)
    out = np.stack([res.results[b]["out"] for b in range(B)]).astype(np.float32)
    if _trace:
        _CACHED["exec_time_ns"] = res.exec_time_ns
    return out


# revision 23
# speedup vs baseline: 1.0877x; 1.0877x over previous
"""Single-head attention on 8 TRN2 NeuronCores — data-parallel over batch.

Reference (per batch element b):
    q = x @ Wq.T + bq; k = x @ Wk.T + bk; v = x @ Wv.T + bv     [S, D]
    scores = q @ k.T / sqrt(S); masked where attention_mask==0
    out = softmax(scores) @ v                                    [S, D]

Shapes: B=8, S=2048, DIN=1024, D=128.  Core i computes batch element i.

Key optimizations over the straightforward version:
  * Key compaction: softmax is permutation-invariant over keys and ~half the
    keys are masked out.  Host-side we gather the unmasked keys and pad to a
    fixed M_pad (multiple of 128).  scores/exp/context then run on M_pad keys
    instead of S.  Padded keys get v=0 (zeroed via mask multiply) and are
    excluded from the softmax denominator by using the 0/1 mask column as the
    stationary operand of the denominator matmuls.
  * fp8 (e4m3) x and 16*Wq for the q projection halve that DMA; the 1/16
    rescale is folded into the bias-apply copy.  The k/v path stays bf16 —
    the near-uniform softmax averages ~1000 v rows with heavy cancellation,
    so fp8 noise in v does not shrink relative to the output.
  * Input DMAs are spread across the sync/gpsimd/vector/scalar queues and
    sized large, since each dma_start costs ~0.7us of issue time on its queue.
  * The middle loop is split into two query halves (i in [0,1024), [1024,2048))
    so the context accumulator fits in 2 PSUM banks and the first half's
    normalize/store tail overlaps the second half's sweep.  Consumer matmuls
    (denominator + context) of step jt are emitted after scores of step jt+1
    so the in-order PE never idles waiting for exp.
  * Denominator matmuls (M=1) for the 2 query chunks of a half are packed into
    different 32-column groups of the PE array via tile_position, sharing one
    PSUM bank across all 4 (half, chunk) pairs.
  * PE warm-up matmuls during the initial DMA engage the HAM clock-gate
    (1.2 -> 2.4 GHz) before the real work starts; more are interleaved into
    the DMA-paced k projection to keep it engaged.
"""

import numpy as np
import ml_dtypes

B, S, DIN, DOUT = 8, 2048, 1024, 128
N_CORES = 8
NCH = DIN // 128          # 8 contraction chunks
BF16 = ml_dtypes.bfloat16
FP8 = ml_dtypes.float8_e4m3fn
SCALE = 1.0 / float(np.sqrt(S))
WSCALE = 16.0             # Wq shipped as 16*Wq in fp8; undone in bias copy

_CACHED = {}


def _blocks(total, maxw=512):
    out = []
    off = 0
    while off < total:
        w = min(maxw, total - off)
        out.append((off, w))
        off += w
    return out


def _build(m_pad):
    import concourse.bacc as bacc
    import concourse.mybir as mybir
    from concourse.tile import TileContext

    dt = mybir.dt
    F32, BF, F8 = dt.float32, dt.bfloat16, dt.float8e4
    Exp = mybir.ActivationFunctionType.Exp
    Ident = mybir.ActivationFunctionType.Identity
    Copy = mybir.ActivationFunctionType.Copy
    MUL, ADD = mybir.AluOpType.mult, mybir.AluOpType.add

    nkt = m_pad // 128            # key tiles
    kblk = _blocks(m_pad)         # column blocks for the k/v projections
    # blob32: bq | bk | (unused) | mcolf | identf | onesrow(128) | bvrow(128)
    nb32 = 3 + nkt + 128 + 256

    nc = bacc.Bacc("TRN2", target_bir_lowering=False)

    # xq[p, ((sb*8+c)*512)+s'] = x[sb*512+s', c*128+p], 4 slabs of s
    xq = [nc.declare_dram_parameter(f"xq{sb}", [128, NCH * 512], F8, False)
          for sb in range(4)]
    # xkv2[p2][p, i*m_pad + j'] = x[idx[j'], (2*p2+i)*128+p]
    xkv = [nc.declare_dram_parameter(f"xkv{p2}", [128, 2 * m_pad], BF, False)
           for p2 in range(4)]
    wkv = nc.declare_dram_parameter("wkv", [128, 2 * NCH * 128], BF, False)
    wq = nc.declare_dram_parameter("wq", [128, NCH * 128], F8, False)
    blob32 = nc.declare_dram_parameter("blob32", [128, nb32], F32, False)
    out = nc.declare_dram_parameter("out", [S, DOUT], F32, True)
    import os
    _dbg = os.environ.get("KDBG")
    if _dbg:
        dbgv = nc.declare_dram_parameter("dbgv", [128, m_pad], F32, True)
        dbgk = nc.declare_dram_parameter("dbgk", [128, m_pad], F32, True)
        dbgq = nc.declare_dram_parameter("dbgq", [128, S], F32, True)

    with TileContext(nc) as tc:
        with (
            tc.tile_pool(name="const", bufs=1) as cp,
            tc.tile_pool(name="work", bufs=1) as wp,
            tc.tile_pool(name="et", bufs=3) as ep,
            tc.tile_pool(name="ctxs", bufs=2) as xp,
            tc.tile_pool(name="io", bufs=4) as iop,
        ):
            # ---- warm scratch (no DMA dependency) ----
            warm = wp.tile([128, 512], BF, tag="warm")
            nc.gpsimd.memset(warm[:], 0.0)
            warmf = wp.tile([128, 16], F32, tag="warmf")
            nc.gpsimd.memset(warmf[:], 0.0)
            # Both HW DGE queues (sync + scalar) pull the k/v-side
            # activations first, interleaved in the order the k/v projection
            # consumes them; q-side follows.  (gpsimd DMA is a ~90 GB/s
            # software DGE — never used for bulk.)
            xkv_sb = [None] * 4
            xq_sb = [None] * 4

            def dma(eng, name, shape, dtp, dram):
                t = cp.tile(shape, dtp, tag=name)
                eng.dma_start(out=t[:], in_=dram[:])
                return t

            xkv_sb[0] = dma(nc.scalar, "xkv0", [128, 2 * m_pad], BF, xkv[0])
            xq_sb[0] = dma(nc.scalar, "xq0", [128, NCH * 512], F8, xq[0])
            xq_sb[1] = dma(nc.scalar, "xq1", [128, NCH * 512], F8, xq[1])
            xkv_sb[2] = dma(nc.scalar, "xkv2", [128, 2 * m_pad], BF, xkv[2])

            wkv_sb = dma(nc.sync, "wkv", [128, 2 * NCH * 128], BF, wkv)
            wq_sb = dma(nc.sync, "wq", [128, NCH * 128], F8, wq)
            b32_sb = dma(nc.sync, "b32", [128, nb32], F32, blob32)
            xkv_sb[1] = dma(nc.sync, "xkv1", [128, 2 * m_pad], BF, xkv[1])
            xkv_sb[3] = dma(nc.sync, "xkv3", [128, 2 * m_pad], BF, xkv[3])
            xq_sb[2] = dma(nc.sync, "xq2", [128, NCH * 512], F8, xq[2])
            xq_sb[3] = dma(nc.sync, "xq3", [128, NCH * 512], F8, xq[3])

            # exp table load AFTER the scalar-queue DMA issues
            warme = wp.tile([128, 16], F32, tag="warme")
            nc.scalar.activation(warme[:], warmf[:], Exp)   # load exp table

            wk_sb = wkv_sb[:, 0:NCH * 128]
            wv_sb = wkv_sb[:, NCH * 128:2 * NCH * 128]
            bq_sb = b32_sb[:, 0:1]
            bk_sb = b32_sb[:, 1:2]
            bvT_sb = b32_sb[:, 2:3]
            mcolf_sb = b32_sb[:, 3:3 + nkt]
            identf_sb = b32_sb[:, 3 + nkt:3 + nkt + 128]
            onesr_sb = b32_sb[0:1, 3 + nkt + 128:3 + nkt + 256]
            bvr_sb = b32_sb[0:1, 3 + nkt + 256:3 + nkt + 384]
            mcol_sb = wp.tile([128, nkt], BF, tag="mcol")

            qT_sb = wp.tile([128, S], BF, tag="qT")
            kT_sb = wp.tile([128, m_pad], BF, tag="kT")
            v_sb = wp.tile([128, m_pad], BF, tag="v")

            def xkv_ap(c, o, w):
                return xkv_sb[c // 2][:, (c % 2) * m_pad + o:
                                      (c % 2) * m_pad + o + w]

            # ================= prologue =================
            with tc.tile_pool(name="proj", bufs=8, space="PSUM") as pp:
                kps = [pp.tile([128, w], F32, tag="proj", name=f"kps{i}")
                       for i, (o, w) in enumerate(kblk)]
                vps = [pp.tile([128, w], F32, tag="proj", name=f"vps{i}")
                       for i, (o, w) in enumerate(kblk)]
                qps = [pp.tile([128, 512], F32, tag="proj", name=f"qps{i}")
                       for i in range(2)]

                # PE warm-up into qps[0] (reset later by the real qh0
                # start=True): engage HAM while DMAs stream
                for _ in range(4):
                    nc.tensor.matmul(qps[0][:], warm[:, 0:128], warm[:],
                                     start=True, stop=True)
                nc.vector.tensor_copy(mcol_sb[:], mcolf_sb[:])  # bf16 cast
                onesr16 = wp.tile([1, 128], BF, tag="onesr16")
                nc.gpsimd.memset(onesr16[:], 1.0)
                bvr16 = wp.tile([1, 128], BF, tag="bvr16")
                nc.vector.tensor_copy(bvr16[:], bvr_sb)   # bf16 cast

                def kv_chunk(c):
                    # kT[d, j'] += wk_c.T @ x_c ; v[j', d] += x_c.T @ wv_c
                    for i, (o, w) in enumerate(kblk):
                        nc.tensor.matmul(
                            kps[i][:], wk_sb[:, c * 128:(c + 1) * 128],
                            xkv_ap(c, o, w),
                            start=(c == 0), stop=(c == NCH - 1))
                    for jt in range(nkt):
                        i, r = divmod(jt, 4)
                        # start only on the first region per PSUM tile: the
                        # lazy zero of start=True covers ALL columns of the
                        # written partitions, so a later region's start would
                        # wipe its neighbors' already-accumulated chunk
                        nc.tensor.matmul(
                            vps[i][:, r * 128:(r + 1) * 128],
                            xkv_ap(c, jt * 128, 128),
                            wv_sb[:, c * 128:(c + 1) * 128],
                            start=(c == 0 and r == 0), stop=False,
                            skip_group_check=True)

                def qh0_group(sb, half):
                    for c in range(4 * half, 4 * half + 4):
                        nc.tensor.matmul(
                            qps[sb][:], wq_sb[:, c * 128:(c + 1) * 128],
                            xq_sb[sb][:, c * 512:(c + 1) * 512],
                            start=(c == 0), stop=(c == NCH - 1))

                kv_chunk(0)
                kv_chunk(1)
                for _ in range(2):
                    nc.tensor.matmul(qps[1][:], warm[:, 0:128], warm[:],
                                     start=True, stop=True)
                qh0_group(0, 0)
                qh0_group(0, 1)
                kv_chunk(2)
                kv_chunk(3)
                qh0_group(1, 0)
                qh0_group(1, 1)
                kv_chunk(4)
                kv_chunk(5)
                kv_chunk(6)
                kv_chunk(7)

                # v bias via K=1 outer product ones[j] x bv[d], then the
                # masked psum->sbuf copies; pads get v=0
                for jt in range(nkt):
                    i, r = divmod(jt, 4)
                    nc.tensor.matmul(
                        vps[i][:, r * 128:(r + 1) * 128],
                        onesr16[:], bvr16[:],
                        start=False, stop=True, skip_group_check=True)

                # projections out of PSUM: kT on ACT, q on DVE, v masked on
                # ACT+DVE alternating
                for i, (o, w) in enumerate(kblk):
                    nc.scalar.activation(kT_sb[:, o:o + w], kps[i][:], Ident,
                                         bias=bk_sb)
                    if i == 0:
                        for _ in range(10):
                            nc.tensor.matmul(kps[0][:], warm[:, 0:128],
                                             warm[:], start=True, stop=True)
                for sb in range(2):
                    nc.vector.tensor_scalar(
                        qT_sb[:, sb * 512:(sb + 1) * 512], qps[sb][:],
                        1.0 / WSCALE, bq_sb, MUL, ADD)
                for jt in range(nkt):
                    i, r = divmod(jt, 4)
                    eng = nc.vector if jt % 2 else nc.scalar
                    if jt % 2:
                        nc.vector.tensor_scalar_mul(
                            v_sb[:, jt * 128:(jt + 1) * 128],
                            vps[i][:, r * 128:(r + 1) * 128],
                            mcolf_sb[:, jt:jt + 1])
                    else:
                        nc.scalar.activation(
                            v_sb[:, jt * 128:(jt + 1) * 128],
                            vps[i][:, r * 128:(r + 1) * 128], Copy,
                            scale=mcolf_sb[:, jt:jt + 1])

            if _dbg:
                dv = wp.tile([128, m_pad], F32, tag="dbgv")
                nc.vector.tensor_copy(dv[:], v_sb[:])
                nc.sync.dma_start(out=dbgv[:], in_=dv[:])
                dk = wp.tile([128, m_pad], F32, tag="dbgk")
                nc.vector.tensor_copy(dk[:], kT_sb[:])
                nc.sync.dma_start(out=dbgk[:], in_=dk[:])
                dq = wp.tile([128, S], F32, tag="dbgq")
                nc.vector.tensor_copy(dq[:], qT_sb[:])
                nc.sync.dma_start(out=dbgq[:], in_=dq[:])

            # ================= middle sweep =================
            with (
                tc.tile_pool(name="sp", bufs=2, space="PSUM") as sp,
                tc.tile_pool(name="cpp", bufs=2, space="PSUM") as cpp,
                tc.tile_pool(name="sums", bufs=1, space="PSUM") as smp,
                tc.tile_pool(name="tp", bufs=1, space="PSUM") as tpp,
            ):
                sums_ps = smp.tile([128, 512], F32, tag="sums")
                ss_sb = wp.tile([128, 512], F32, tag="ss")
                rt_sb = wp.tile([128, 2 * 8], F32, tag="rt")
                recip_sb = wp.tile([128, 2 * 8], F32, tag="recip")

                qh1_state = {}

                def qh1_piece(sb, cpair):
                    # second-half q projection, 2 chunks at a time in the tp
                    # bank; bias copy on DVE so ACT keeps streaming exps
                    if sb not in qh1_state:
                        qh1_state[sb] = tpp.tile([128, 512], F32, tag="tp",
                                                 name=f"qh1_{sb}")
                    qp = qh1_state[sb]
                    for c in (2 * cpair, 2 * cpair + 1):
                        nc.tensor.matmul(
                            qp[:], wq_sb[:, c * 128:(c + 1) * 128],
                            xq_sb[sb][:, c * 512:(c + 1) * 512],
                            start=(c == 0), stop=(c == NCH - 1))
                    if cpair == 3:
                        nc.vector.tensor_scalar(
                            qT_sb[:, sb * 512:(sb + 1) * 512], qp[:],
                            1.0 / WSCALE, bq_sb, MUL, ADD)

                def emit_scores(h, jt):
                    ps = sp.tile([128, 1024], F32, tag="sc", name=f"sc{h}_{jt}")
                    for n in range(2):
                        nc.tensor.matmul(
                            ps[:, n * 512:(n + 1) * 512],
                            kT_sb[:, jt * 128:(jt + 1) * 128],
                            qT_sb[:, h * 1024 + n * 512:
                                  h * 1024 + (n + 1) * 512],
                            start=True, stop=True)
                    et = ep.tile([128, 1024], BF, tag="et", name=f"et{h}_{jt}")
                    nc.scalar.activation(et[:], ps[:], Exp, scale=SCALE)
                    return et

                def emit_consume(h, jt, et, ctx_ps):
                    for icg in range(2):
                        base = 64 * h + 32 * icg
                        nc.tensor.matmul(
                            sums_ps[base:base + 1, :],
                            mcol_sb[:, jt:jt + 1],
                            et[:, icg * 512:(icg + 1) * 512],
                            start=(jt == 0), stop=(jt == nkt - 1),
                            tile_position=(0, base),
                            skip_group_check=True)
                    for n in range(2):
                        nc.tensor.matmul(
                            ctx_ps[n][:],
                            v_sb[:, jt * 128:(jt + 1) * 128],
                            et[:, n * 512:(n + 1) * 512],
                            start=(jt == 0), stop=(jt == nkt - 1))

                tail_state = {}

                def tail_stage(h, ctx_ps, stage):
                    """Stage 0: denominator chain + ctx psum drain;
                    1: first 512 ctx cols transposed/normalized/stored;
                    2: second 512.  For h=0 (overlapping the h1 sweep) all
                    copies stay off ACT so exp keeps streaming; for the final
                    h=1 tail both ACT and DVE are used for latency."""
                    st = tail_state.setdefault(h, {})
                    if stage == 0:
                        if h == 0:
                            st['t'] = [(tpp.tile([128, 512], F32, tag="tp",
                                                 name="t0a"), 0)] * 3
                        else:
                            ta = sp.tile([128, 1024], F32, tag="sc", name="t1a")
                            tb = sp.tile([128, 1024], F32, tag="sc", name="t1b")
                            st['t'] = [(ta, 0), (ta, 4), (tb, 0)]
                        # denominator chain first (recip gates the normalizes)
                        if h == 0:
                            nc.vector.tensor_copy(
                                ss_sb[0:64, :], sums_ps[0:64, :])
                        else:
                            nc.scalar.activation(
                                ss_sb[64:128, :], sums_ps[64:128, :], Copy)
                        ctx_sb = xp.tile([128, 1024], F32, tag="ctxT",
                                         name=f"ctxT{h}")
                        st['ctx'] = ctx_sb
                        nc.vector.tensor_copy(ctx_sb[:, 0:512], ctx_ps[0][:])
                        stile, soff = st['t'][0]
                        for b in range(4):
                            tcol = (soff + b) * 128
                            nc.tensor.transpose(
                                stile[:, tcol:tcol + 128],
                                ss_sb[:, b * 128:(b + 1) * 128], identf_sb)
                        # one strided copy pulls all 8 denominator columns
                        # (avoids per-transpose PE-write/DVE-read bank
                        # serialization)
                        ein = stile[:, soff * 128:soff * 128 + 512].rearrange(
                            "p (b i x) -> p b i x", b=4, i=4)
                        ein = ein[:, :, 2 * h:2 * h + 2, 0:1]
                        eout = rt_sb[:, h * 8:h * 8 + 8].rearrange(
                            "p (i b) -> p b i", i=2).unsqueeze(-1)
                        nc.vector.tensor_copy(eout, ein)
                        nc.vector.reciprocal(
                            recip_sb[:, h * 8:(h + 1) * 8],
                            rt_sb[:, h * 8:(h + 1) * 8])
                        if h == 0:
                            nc.vector.tensor_copy(ctx_sb[:, 512:1024],
                                                  ctx_ps[1][:])
                        else:
                            nc.scalar.activation(ctx_sb[:, 512:1024],
                                                 ctx_ps[1][:], Copy)
                        return
                    icg = stage - 1
                    ctx_sb = st['ctx']
                    ctile, coff = st['t'][1 + icg]
                    o_sb = iop.tile([128, 512], F32, tag="osb",
                                    name=f"osb{h}_{icg}")
                    for b in range(4):
                        tcol = (coff + b) * 128
                        nc.tensor.transpose(
                            ctile[:, tcol:tcol + 128],
                            ctx_sb[:, icg * 512 + b * 128:
                                   icg * 512 + (b + 1) * 128], identf_sb)
                    for b in range(4):
                        tcol = (coff + b) * 128
                        it = h * 8 + icg * 4 + b
                        if h == 1 and b % 2 == 0:
                            nc.scalar.activation(
                                o_sb[:, b * 128:(b + 1) * 128],
                                ctile[:, tcol:tcol + 128], Copy,
                                scale=recip_sb[:, it:it + 1])
                        else:
                            nc.vector.tensor_scalar_mul(
                                o_sb[:, b * 128:(b + 1) * 128],
                                ctile[:, tcol:tcol + 128],
                                recip_sb[:, it:it + 1])
                    r0 = h * 1024 + icg * 512
                    eng = nc.scalar if (h == 1 and icg == 1) else nc.sync
                    eng.dma_start(
                        out=out[r0:r0 + 512, :].rearrange(
                            "(b p) d -> p b d", b=4),
                        in_=o_sb[:])

                ctx_tiles = {}

                def ctx_tile(h):
                    if h not in ctx_tiles:
                        ctx_tiles[h] = [
                            cpp.tile([128, 512], F32, tag="ctx",
                                     name=f"ctx{h}_{icg}")
                            for icg in range(2)]
                    return ctx_tiles[h]

                for h in range(2):
                    prev = None
                    for jt in range(nkt):
                        et = emit_scores(h, jt)
                        if h == 1 and jt in (1, 2, 3):
                            # h0 tail staged here: its PE work hides in the
                            # exp pipeline-fill bubble of the h1 sweep
                            tail_stage(0, ctx_tile(0), jt - 1)
                        if prev is not None:
                            emit_consume(h, jt - 1, prev, ctx_tile(h))
                        prev = et
                        if h == 0 and jt <= 3:
                            qh1_piece(2, jt)
                        elif h == 0 and 4 <= jt <= 7:
                            qh1_piece(3, jt - 4)
                    emit_consume(h, nkt - 1, prev, ctx_tile(h))
                for stage in range(3):
                    tail_stage(1, ctx_tile(1), stage)

    nc.compile()
    return nc


def _prep_core_inputs(xb, Wq, bq, Wk, bk, Wv, bv, maskb, m_pad):
    """Host-side layout prep for one batch element."""
    nkt = m_pad // 128
    idx = np.nonzero(maskb != 0)[0]
    cnt = len(idx)

    xg = np.zeros((m_pad, DIN), dtype=np.float32)
    xg[:cnt] = xb[idx]

    def chunkT(m, dtp):  # [rows, DIN] -> [128, NCH*rows], m.T chunked over DIN
        mt = np.ascontiguousarray(m.T)          # [DIN, rows]
        c = mt.shape[1]
        return np.ascontiguousarray(
            mt.reshape(NCH, 128, c).transpose(1, 0, 2).reshape(128, NCH * c)
        ).astype(dtp)

    xt = xb.T.reshape(NCH, 128, 4, 512)          # [c, p, sb, s']
    xq4 = np.ascontiguousarray(
        xt.transpose(1, 2, 0, 3).reshape(128, 4 * NCH * 512)).astype(FP8)

    xkvt = xg.T.reshape(NCH, 128, m_pad)         # [c, p, j']
    m01 = np.zeros((m_pad,), dtype=np.float32)
    m01[:cnt] = 1.0
    mt = np.ascontiguousarray(m01.reshape(nkt, 128).T)   # [128, nkt]

    identf = np.eye(128, dtype=np.float32)
    onesrow = np.zeros((128, 128), dtype=np.float32)
    onesrow[0, :] = 1.0
    bvrow = np.zeros((128, 128), dtype=np.float32)
    bvrow[0, :] = bv
    blob32 = np.concatenate([
        bq.reshape(128, 1), bk.reshape(128, 1), bv.reshape(128, 1),
        mt, identf, onesrow, bvrow], axis=1).astype(np.float32)

    d = {
        "wq": chunkT(Wq * WSCALE, FP8),
        "wkv": np.concatenate([chunkT(Wk, BF16), chunkT(Wv, BF16)], axis=1),
        "blob32": np.ascontiguousarray(blob32),
    }
    for sb in range(4):
        d[f"xq{sb}"] = np.ascontiguousarray(
            xq4[:, sb * NCH * 512:(sb + 1) * NCH * 512])
    for p2 in range(4):
        d[f"xkv{p2}"] = np.ascontiguousarray(
            np.concatenate([xkvt[2 * p2], xkvt[2 * p2 + 1]], axis=1)
        ).astype(BF16)
    return d


def kernel(x, Wq, bq, Wk, bk, Wv, bv, attention_mask, _trace=False):
    from concourse.bass_utils import run_bass_kernel_spmd

    x = np.asarray(x, dtype=np.float32)
    Wq = np.asarray(Wq, dtype=np.float32)
    Wk = np.asarray(Wk, dtype=np.float32)
    Wv = np.asarray(Wv, dtype=np.float32)
    bq = np.asarray(bq, dtype=np.float32)
    bk = np.asarray(bk, dtype=np.float32)
    bv = np.asarray(bv, dtype=np.float32)
    mask = np.asarray(attention_mask)

    counts = [int((mask[b, 0] != 0).sum()) for b in range(B)]
    m_pad = min(S, max(512, int(-(-max(counts) // 128) * 128)))

    if ("nc", m_pad) not in _CACHED:
        _CACHED[("nc", m_pad)] = _build(m_pad)
    nc = _CACHED[("nc", m_pad)]

    in_maps = [
        _prep_core_inputs(x[b], Wq, bq, Wk, bk, Wv, bv, mask[b, 0], m_pad)
        for b in range(B)
    ]
    res = run_bass_kernel_spmd(
        nc, in_maps, core_ids=list(range(N_CORES)), trace=_trace)
    out = np.stack([res.results[b]["out"] for b in range(B)]).astype(np.float32)
    if _trace:
        _CACHED["exec_time_ns"] = res.exec_time_ns
    return out
